# revision 36
# baseline (speedup 1.0000x reference)
"""MoE (top-2 of 8 experts, SwiGLU) Trainium2 kernel.

Strategy (expert parallelism, per the sharding hint):
  - Host: compute router logits/top-2/softmax (0.065% of total FLOPs),
    dispatch tokens to experts (the host-side all-to-all "dispatch").
  - Device: 8 NeuronCores, core e runs expert e's SwiGLU FFN over the
    tokens routed to it (padded to a shared capacity C). All matmuls in
    bf16 with fp32 PSUM accumulation; weights SBUF-resident.
  - Host: weighted scatter-add combine (the "combine" step).

Device compute per core (transposed so every matmul uses natural,
transpose-free operand layouts; PSUM accumulates over the contraction):
  hT[h_chunk, tok] = wg.T @ xt   (accumulate K=D over 8 chunks of 128)
  h2 = silu(hT_gate) * hT_up     (ACT silu + DVE mul, bf16 out)
  yT[d_chunk, tok] = wd.T @ h2   (accumulate K=H over 16 chunks of 128)

Weights are host-prepacked to [p=128, out_block, k_chunk, 128] so each
128-column weight block is one contiguous DMA; the per-block DMA split
lets the first matmuls start ~5us in instead of waiting ~45us for the
full 12.6MB weight load.
"""

import sys

if "/opt/trn_rl_repo" not in sys.path:
    sys.path.insert(0, "/opt/trn_rl_repo")

import ml_dtypes
import numpy as np

NUM_EXPERTS = 8
TOP_K = 2
EMB = 1024
HID = 2048
P = 128
KD = EMB // P  # 8
KH = HID // P  # 16
TOK = 512  # main token tile (one PSUM bank of f32)

_BF16 = ml_dtypes.bfloat16


def _make_tile_context(nc):
    """TileContext whose emitted instructions carry at most ONE sem wait.

    The walrus codegen bundled in this environment rejects any instruction
    with more than one sync-wait command ("Too many sync wait commands").
    Tile's scheduler freely attaches several waits to one instruction (and
    its exit drain waits on every frontier semaphore), so hoist all but the
    last wait onto dedicated same-engine NoOps immediately preceding the
    instruction.
    """
    import concourse.mybir as mybir
    import concourse.tile as tile
    from concourse.vector_clock import ScopedClock

    class OneWaitTC(tile.TileContext):
        def _split_waits(self, inst):
            si = getattr(inst, "sync_info", None)
            if si is None or not si.on_wait or len(si.on_wait) <= 1:
                return
            engine = getattr(inst, "engine", None)
            if engine is None or engine == mybir.EngineType.Unassigned:
                return
            waits = list(si.on_wait)
            for w in waits[:-1]:
                nop = mybir.InstNoOp(
                    name=self.nc.get_next_instruction_name(),
                    sync_info=mybir.SyncInfo(on_wait=[w], on_update=[]),
                    bass_nofuse=True,
                    engine=engine,
                )
                super()._commit_instruction(nop, lazy_reg_writes=False)
            inst.sync_info = mybir.SyncInfo(
                on_wait=[waits[-1]], on_update=list(si.on_update or [])
            )

        def _commit_instruction(self, inst, lazy_reg_writes: bool = True):
            if isinstance(inst, mybir.Instruction):
                self._split_waits(inst)
            super()._commit_instruction(inst, lazy_reg_writes)

        def _drain_and_barrier(self, tick_clock, wait_clock):
            nc = self.nc
            drain_inst = nc.sync.drain()
            wait_clock.add_sem_waits(
                drain_inst.ins, ScopedClock({None: tick_clock.global_clock})
            )
            si = drain_inst.ins.sync_info
            if si is not None and si.on_wait and len(si.on_wait) > 1:
                waits = list(si.on_wait)
                drain_inst.ins.sync_info = mybir.SyncInfo(
                    on_wait=waits[:1], on_update=list(si.on_update or [])
                )
                # spread the remaining frontier waits across engines so they
                # retire in parallel instead of serializing on SP
                engines = [nc.sync, nc.tensor, nc.vector, nc.scalar, nc.gpsimd]
                for i, w in enumerate(waits[1:]):
                    d2 = engines[i % len(engines)].drain()
                    d2.ins.sync_info = mybir.SyncInfo(on_wait=[w], on_update=[])
            nc.all_engine_barrier()
            assert self.sems is not None
            popped = nc._tile_sem_poison_stack.pop()
            assert popped is self._sem_poison
            nc.clear_and_free_semaphores(list(self.sems.allocated().values()))
            nc.all_engine_barrier()

    return OneWaitTC(nc)


def token_tiles(C: int):
    tiles = [TOK] * (C // TOK)
    if C % TOK:
        tiles.append(C % TOK)
    return tiles


def build_moe_expert_kernel(C: int):
    """One SPMD program: SwiGLU FFN of a single expert over C tokens."""
    import concourse.bass as bass
    import concourse.mybir as mybir

    dt = mybir.dt
    nc = bass.Bass()

    # prepacked layouts (see pack_* helpers below); xt is packed per token
    # tile ([P, KD*tok] blocks) so each tile's DMA is one contiguous
    # 8KB-per-partition read instead of 8 strided 1KB lines
    xt = nc.dram_tensor("xt", [P, C * KD], dt.bfloat16, kind="ExternalInput")
    wg = nc.dram_tensor("wg", [P, KH, KD, P], dt.bfloat16, kind="ExternalInput")
    wu = nc.dram_tensor("wu", [P, KH, KD, P], dt.bfloat16, kind="ExternalInput")
    wd = nc.dram_tensor("wd", [P, KD, KH, P], dt.bfloat16, kind="ExternalInput")
    yt = nc.dram_tensor("yt", [P, KD, C], dt.float32, kind="ExternalOutput")

    tiles = token_tiles(C)

    with _make_tile_context(nc) as tc:
        with (
            tc.tile_pool(name="weights", bufs=1) as wpool,
            tc.tile_pool(name="xin", bufs=3) as xpool,
            tc.tile_pool(name="h2", bufs=2) as hpool,
            tc.tile_pool(name="sg", bufs=4) as spool,
            tc.tile_pool(name="out", bufs=4) as opool,
            # one shared pool: all psum tiles rotate through all 8 banks,
            # maximizing the slot-recycle distance the matmul group starts
            # wait on
            tc.tile_pool(name="ps", bufs=8, space="PSUM") as psA,
        ):
            # one tile per 128-col weight block: tiles are Tile's dependency
            # unit, so the m=0 matmuls only wait for their own 512KB block
            wg_sb = [
                wpool.tile([P, KD, P], dt.bfloat16, tag=f"wg{m}", name=f"wg{m}")
                for m in range(KH)
            ]
            wu_sb = [
                wpool.tile([P, KD, P], dt.bfloat16, tag=f"wu{m}", name=f"wu{m}")
                for m in range(KH)
            ]
            wd_sb = [
                wpool.tile([P, KH, P], dt.bfloat16, tag=f"wd{m2}", name=f"wd{m2}")
                for m2 in range(KD)
            ]
            # token loads ride the (otherwise idle) GpSimd SWDGE path so they
            # never queue behind the 12.6MB of weights on the SP HWDGE ring.
            # The first tile is the kernel's critical path: it goes on the
            # empty ACT HWDGE ring (~4us for 1MB vs ~10us on SWDGE); only the
            # first — later ACT DMAs would stall silu work on pool-slot waits.
            xt_tiles = []
            off = 0
            for t_i, tok in enumerate(tiles):
                src = xt[:, off * KD : (off + tok) * KD].rearrange(
                    "p (k t) -> p k t", k=KD
                )
                if t_i == 0:
                    # separate half-tiles on the ACT ring: tiles are the
                    # dependency unit, so the k<4 matmuls of the first group
                    # only wait for the first 512KB half
                    half = KD // 2
                    xt0a = xpool.tile(
                        [P, half, TOK], dt.bfloat16, tag="xt0a", name="xt0a"
                    )
                    xt0b = xpool.tile(
                        [P, KD - half, TOK], dt.bfloat16, tag="xt0b", name="xt0b"
                    )
                    nc.scalar.dma_start(xt0a[:, :, :tok], src[:, :half])
                    nc.scalar.dma_start(xt0b[:, :, :tok], src[:, half:])
                    xt_tiles.append((xt0a, xt0b))
                else:
                    xt_sb = xpool.tile(
                        [P, KD, TOK], dt.bfloat16, tag="xt", name=f"xt{off}"
                    )
                    nc.gpsimd.dma_start(xt_sb[:, :, :tok], src)
                    xt_tiles.append(xt_sb)
                off += tok

            for m in range(KH):
                nc.sync.dma_start(wg_sb[m][:], wg[:, m])
                nc.sync.dma_start(wu_sb[m][:], wu[:, m])
            for m2 in range(KD):
                nc.sync.dma_start(wd_sb[m2][:], wd[:, m2])

            off = 0
            for t_i, tok in enumerate(tiles):
                ts = slice(off, off + tok)
                off += tok
                xt_sb = xt_tiles[t_i]
                if t_i == 0:
                    a, b = xt_sb
                    half = a.shape[1]
                    rhs = lambda k: (a[:, k] if k < half else b[:, k - half])
                else:
                    rhs = lambda k, _x=xt_sb: _x[:, k]

                h2_sb = hpool.tile([P, KH, TOK], dt.bfloat16, tag="h2")
                for m in range(KH):
                    pg = psA.tile([P, TOK], dt.float32, tag="ps", name=f"pg{off}_{m}")
                    for k in range(KD):
                        nc.tensor.matmul(
                            pg[:, :tok], wg_sb[m][:, k], rhs(k)[:, :tok],
                            start=(k == 0), stop=(k == KD - 1),
                        )
                    pu = psA.tile([P, TOK], dt.float32, tag="ps", name=f"pu{off}_{m}")
                    for k in range(KD):
                        nc.tensor.matmul(
                            pu[:, :tok], wu_sb[m][:, k], rhs(k)[:, :tok],
                            start=(k == 0), stop=(k == KD - 1),
                        )
                    # fast DVE copies release the PSUM banks immediately;
                    # silu+mul then run off SBUF, off the bank-recycle path
                    pgs = spool.tile([P, TOK], dt.float32, tag="pgs")
                    nc.vector.tensor_copy(pgs[:, :tok], pg[:, :tok])
                    pus = spool.tile([P, TOK], dt.float32, tag="pus")
                    nc.vector.tensor_copy(pus[:, :tok], pu[:, :tok])
                    sg = spool.tile([P, TOK], dt.bfloat16, tag="sg")
                    nc.scalar.activation(
                        sg[:, :tok], pgs[:, :tok],
                        mybir.ActivationFunctionType.Silu,
                    )
                    nc.vector.tensor_mul(
                        h2_sb[:, m, :tok], sg[:, :tok], pus[:, :tok]
                    )

                for m2 in range(KD):
                    py = psA.tile([P, TOK], dt.float32, tag="ps", name=f"py{off}_{m2}")
                    for k2 in range(KH):
                        nc.tensor.matmul(
                            py[:, :tok], wd_sb[m2][:, k2], h2_sb[:, k2, :tok],
                            start=(k2 == 0), stop=(k2 == KH - 1),
                        )
                    ot = opool.tile([P, TOK], dt.float32, tag="ot")
                    nc.vector.tensor_copy(ot[:, :tok], py[:, :tok])
                    # SWDGE, not the SP HWDGE ring: each output burst on the
                    # SP ring steals a PE sequencer ifetch slot (~432ns stall
                    # per output DMA observed)
                    nc.gpsimd.dma_start(yt[:, m2, ts], ot[:, :tok])

    return nc


def pack_lhsT(w: np.ndarray) -> np.ndarray:
    """[K, M] weight -> [p=128, m_block, k_chunk, 128] bf16, so that
    slice [:, m, k, :] is the lhsT tile for contraction chunk k, output
    block m, and each [:, m] block is one contiguous DMA."""
    K, M = w.shape
    kc, mb = K // P, M // P
    return np.ascontiguousarray(
        w.reshape(kc, P, mb, P).transpose(1, 2, 0, 3)
    ).astype(_BF16)


def pack_tokens(xe: np.ndarray, C: int) -> np.ndarray:
    """[n, D] tokens -> zero-padded [p=128, C*KD] bf16, blocked per token
    tile as [KD, tok] per partition (one contiguous DMA per tile)."""
    n = xe.shape[0]
    out = np.zeros((P, C * KD), dtype=_BF16)
    off = 0
    for tok in token_tiles(C):
        xe_t = xe[off : min(off + tok, n)]
        nt = xe_t.shape[0]
        if nt:
            blk = np.zeros((P, KD, tok), dtype=_BF16)
            # [nt, D] -> [D, nt] -> [KD, P, nt] -> [P, KD, nt]
            blk[:, :, :nt] = (
                xe_t.T.reshape(KD, P, nt).transpose(1, 0, 2).astype(_BF16)
            )
            out[:, off * KD : (off + tok) * KD] = blk.reshape(P, KD * tok)
        off += tok
    return out


def route_tokens(xf: np.ndarray, router_w: np.ndarray):
    """Top-2 routing identical to the reference (softmax over selected)."""
    logits = xf @ router_w  # [T, E]
    # top-2 per token (order irrelevant: softmax over the pair + scatter)
    top_idx = np.argpartition(-logits, TOP_K, axis=-1)[:, :TOP_K]
    tv = np.take_along_axis(logits, top_idx, axis=-1)
    tv = tv - tv.max(axis=-1, keepdims=True)
    ev = np.exp(tv)
    probs = ev / ev.sum(axis=-1, keepdims=True)

    idx, scale = [], []
    for e in range(NUM_EXPERTS):
        hit = top_idx == e  # [T, 2]
        rows = np.nonzero(hit.any(axis=-1))[0]
        w = np.where(hit[rows, 0], probs[rows, 0], probs[rows, 1])
        idx.append(rows)
        scale.append(w.astype(np.float32))
    return idx, scale


def prepare_in_maps(x, router_w, w_gate, w_up, w_down):
    x = np.asarray(x, dtype=np.float32)
    xf = x.reshape(-1, EMB)
    idx, scale = route_tokens(xf, np.asarray(router_w, dtype=np.float32))
    C = max(len(r) for r in idx)  # exact shared capacity

    in_maps = []
    for e in range(NUM_EXPERTS):
        in_maps.append(
            {
                "xt": pack_tokens(xf[idx[e]], C),
                "wg": pack_lhsT(np.asarray(w_gate[e], dtype=np.float32)),
                "wu": pack_lhsT(np.asarray(w_up[e], dtype=np.float32)),
                "wd": pack_lhsT(np.asarray(w_down[e], dtype=np.float32)),
            }
        )
    return in_maps, idx, scale, C, xf


def kernel(x, router_w, w_gate, w_up, w_down):
    from concourse.bass_utils import run_bass_kernel_spmd

    in_maps, idx, scale, C, xf = prepare_in_maps(
        x, router_w, w_gate, w_up, w_down
    )
    nc = build_moe_expert_kernel(C)
    res = None
    last_exc = None
    for _attempt in range(3):
        try:
            res = run_bass_kernel_spmd(nc, in_maps, list(range(NUM_EXPERTS)))
            break
        except Exception as exc:  # transient device wedge: retry
            last_exc = exc
    if res is None:
        raise last_exc

    out = np.zeros_like(xf)
    for e in range(NUM_EXPERTS):
        ytc = np.asarray(res.results[e]["yt"], dtype=np.float32)  # [P, KD, C]
        n = len(idx[e])
        y = ytc.transpose(1, 0, 2).reshape(EMB, C)[:, :n]  # [D, n]
        # indices within one expert are unique -> fancy += is safe
        out[idx[e]] += y.T * scale[e][:, None]
    return out.reshape(np.asarray(x).shape)


# revision 37
# speedup vs baseline: 1.0087x; 1.0087x over previous
"""MoE (top-2 of 8 experts, SwiGLU) Trainium2 kernel.

Strategy (expert parallelism, per the sharding hint):
  - Host: compute router logits/top-2/softmax (0.065% of total FLOPs),
    dispatch tokens to experts (the host-side all-to-all "dispatch").
  - Device: 8 NeuronCores, core e runs expert e's SwiGLU FFN over the
    tokens routed to it (padded to a shared capacity C). All matmuls in
    bf16 with fp32 PSUM accumulation; weights SBUF-resident.
  - Host: weighted scatter-add combine (the "combine" step).

Device compute per core (transposed so every matmul uses natural,
transpose-free operand layouts; PSUM accumulates over the contraction):
  hT[h_chunk, tok] = wg.T @ xt   (accumulate K=D over 8 chunks of 128)
  h2 = silu(hT_gate) * hT_up     (ACT silu + DVE mul, bf16 out)
  yT[d_chunk, tok] = wd.T @ h2   (accumulate K=H over 16 chunks of 128)

Weights are host-prepacked to [p=128, out_block, k_chunk, 128] so each
128-column weight block is one contiguous DMA; the per-block DMA split
lets the first matmuls start ~5us in instead of waiting ~45us for the
full 12.6MB weight load.
"""

import sys

if "/opt/trn_rl_repo" not in sys.path:
    sys.path.insert(0, "/opt/trn_rl_repo")

import ml_dtypes
import numpy as np

NUM_EXPERTS = 8
TOP_K = 2
EMB = 1024
HID = 2048
P = 128
KD = EMB // P  # 8
KH = HID // P  # 16
TOK = 512  # main token tile (one PSUM bank of f32)

_BF16 = ml_dtypes.bfloat16


def _make_tile_context(nc):
    """TileContext whose emitted instructions carry at most ONE sem wait.

    The walrus codegen bundled in this environment rejects any instruction
    with more than one sync-wait command ("Too many sync wait commands").
    Tile's scheduler freely attaches several waits to one instruction (and
    its exit drain waits on every frontier semaphore), so hoist all but the
    last wait onto dedicated same-engine NoOps immediately preceding the
    instruction.
    """
    import concourse.mybir as mybir
    import concourse.tile as tile
    from concourse.vector_clock import ScopedClock

    class OneWaitTC(tile.TileContext):
        def _split_waits(self, inst):
            si = getattr(inst, "sync_info", None)
            if si is None or not si.on_wait or len(si.on_wait) <= 1:
                return
            engine = getattr(inst, "engine", None)
            if engine is None or engine == mybir.EngineType.Unassigned:
                return
            waits = list(si.on_wait)
            for w in waits[:-1]:
                nop = mybir.InstNoOp(
                    name=self.nc.get_next_instruction_name(),
                    sync_info=mybir.SyncInfo(on_wait=[w], on_update=[]),
                    bass_nofuse=True,
                    engine=engine,
                )
                super()._commit_instruction(nop, lazy_reg_writes=False)
            inst.sync_info = mybir.SyncInfo(
                on_wait=[waits[-1]], on_update=list(si.on_update or [])
            )

        def _commit_instruction(self, inst, lazy_reg_writes: bool = True):
            if isinstance(inst, mybir.Instruction):
                self._split_waits(inst)
            super()._commit_instruction(inst, lazy_reg_writes)

        def _drain_and_barrier(self, tick_clock, wait_clock):
            nc = self.nc
            drain_inst = nc.sync.drain()
            wait_clock.add_sem_waits(
                drain_inst.ins, ScopedClock({None: tick_clock.global_clock})
            )
            si = drain_inst.ins.sync_info
            if si is not None and si.on_wait and len(si.on_wait) > 1:
                waits = list(si.on_wait)
                drain_inst.ins.sync_info = mybir.SyncInfo(
                    on_wait=waits[:1], on_update=list(si.on_update or [])
                )
                # spread the remaining frontier waits across engines so they
                # retire in parallel instead of serializing on SP
                engines = [nc.sync, nc.tensor, nc.vector, nc.scalar, nc.gpsimd]
                for i, w in enumerate(waits[1:]):
                    d2 = engines[i % len(engines)].drain()
                    d2.ins.sync_info = mybir.SyncInfo(on_wait=[w], on_update=[])
            nc.all_engine_barrier()
            assert self.sems is not None
            popped = nc._tile_sem_poison_stack.pop()
            assert popped is self._sem_poison
            nc.clear_and_free_semaphores(list(self.sems.allocated().values()))
            nc.all_engine_barrier()

    return OneWaitTC(nc)


def token_tiles(C: int):
    tiles = [TOK] * (C // TOK)
    if C % TOK:
        tiles.append(C % TOK)
    return tiles


def build_moe_expert_kernel(C: int):
    """One SPMD program: SwiGLU FFN of a single expert over C tokens."""
    import concourse.bass as bass
    import concourse.mybir as mybir

    dt = mybir.dt
    nc = bass.Bass()

    # prepacked layouts (see pack_* helpers below); xt is packed per token
    # tile ([P, KD*tok] blocks) so each tile's DMA is one contiguous
    # 8KB-per-partition read instead of 8 strided 1KB lines
    xt = nc.dram_tensor("xt", [P, C * KD], dt.bfloat16, kind="ExternalInput")
    wg = nc.dram_tensor("wg", [P, KH, KD, P], dt.bfloat16, kind="ExternalInput")
    wu = nc.dram_tensor("wu", [P, KH, KD, P], dt.bfloat16, kind="ExternalInput")
    wd = nc.dram_tensor("wd", [P, KD, KH, P], dt.bfloat16, kind="ExternalInput")
    yt = nc.dram_tensor("yt", [P, KD, C], dt.float32, kind="ExternalOutput")

    tiles = token_tiles(C)

    with _make_tile_context(nc) as tc:
        with (
            tc.tile_pool(name="weights", bufs=1) as wpool,
            tc.tile_pool(name="xin", bufs=3) as xpool,
            tc.tile_pool(name="h2", bufs=2) as hpool,
            tc.tile_pool(name="sg", bufs=4) as spool,
            tc.tile_pool(name="out", bufs=4) as opool,
            # one shared pool: all psum tiles rotate through all 8 banks,
            # maximizing the slot-recycle distance the matmul group starts
            # wait on
            tc.tile_pool(name="ps", bufs=8, space="PSUM") as psA,
        ):
            # one tile per 128-col weight block: tiles are Tile's dependency
            # unit, so the m=0 matmuls only wait for their own 512KB block
            wg_sb = [
                wpool.tile([P, KD, P], dt.bfloat16, tag=f"wg{m}", name=f"wg{m}")
                for m in range(KH)
            ]
            wu_sb = [
                wpool.tile([P, KD, P], dt.bfloat16, tag=f"wu{m}", name=f"wu{m}")
                for m in range(KH)
            ]
            wd_sb = [
                wpool.tile([P, KH, P], dt.bfloat16, tag=f"wd{m2}", name=f"wd{m2}")
                for m2 in range(KD)
            ]
            # token loads ride the (otherwise idle) GpSimd SWDGE path so they
            # never queue behind the 12.6MB of weights on the SP HWDGE ring.
            # The first tile is the kernel's critical path: it goes on the
            # empty ACT HWDGE ring (~4us for 1MB vs ~10us on SWDGE); only the
            # first — later ACT DMAs would stall silu work on pool-slot waits.
            xt_tiles = []
            off = 0
            for t_i, tok in enumerate(tiles):
                src = xt[:, off * KD : (off + tok) * KD].rearrange(
                    "p (k t) -> p k t", k=KD
                )
                if t_i == 0:
                    # separate half-tiles on the ACT ring: tiles are the
                    # dependency unit, so the k<4 matmuls of the first group
                    # only wait for the first 512KB half
                    half = KD // 2
                    xt0a = xpool.tile(
                        [P, half, TOK], dt.bfloat16, tag="xt0a", name="xt0a"
                    )
                    xt0b = xpool.tile(
                        [P, KD - half, TOK], dt.bfloat16, tag="xt0b", name="xt0b"
                    )
                    nc.scalar.dma_start(xt0a[:, :, :tok], src[:, :half])
                    nc.scalar.dma_start(xt0b[:, :, :tok], src[:, half:])
                    xt_tiles.append((xt0a, xt0b))
                else:
                    xt_sb = xpool.tile(
                        [P, KD, TOK], dt.bfloat16, tag="xt", name=f"xt{off}"
                    )
                    nc.gpsimd.dma_start(xt_sb[:, :, :tok], src)
                    xt_tiles.append(xt_sb)
                off += tok

            for m in range(KH):
                nc.sync.dma_start(wg_sb[m][:], wg[:, m])
                nc.sync.dma_start(wu_sb[m][:], wu[:, m])
            for m2 in range(KD):
                nc.sync.dma_start(wd_sb[m2][:], wd[:, m2])

            off = 0
            for t_i, tok in enumerate(tiles):
                ts = slice(off, off + tok)
                off += tok
                xt_sb = xt_tiles[t_i]
                if t_i == 0:
                    a, b = xt_sb
                    half = a.shape[1]
                    rhs = lambda k: (a[:, k] if k < half else b[:, k - half])
                else:
                    rhs = lambda k, _x=xt_sb: _x[:, k]

                h2_sb = hpool.tile([P, KH, TOK], dt.bfloat16, tag="h2")
                for m in range(KH):
                    pg = psA.tile([P, TOK], dt.float32, tag="ps", name=f"pg{off}_{m}")
                    for k in range(KD):
                        nc.tensor.matmul(
                            pg[:, :tok], wg_sb[m][:, k], rhs(k)[:, :tok],
                            start=(k == 0), stop=(k == KD - 1),
                        )
                    pu = psA.tile([P, TOK], dt.float32, tag="ps", name=f"pu{off}_{m}")
                    for k in range(KD):
                        nc.tensor.matmul(
                            pu[:, :tok], wu_sb[m][:, k], rhs(k)[:, :tok],
                            start=(k == 0), stop=(k == KD - 1),
                        )
                    # fast DVE copies release the PSUM banks immediately;
                    # silu+mul then run off SBUF, off the bank-recycle path
                    pgs = spool.tile([P, TOK], dt.float32, tag="pgs")
                    nc.vector.tensor_copy(pgs[:, :tok], pg[:, :tok])
                    pus = spool.tile([P, TOK], dt.float32, tag="pus")
                    nc.vector.tensor_copy(pus[:, :tok], pu[:, :tok])
                    sg = spool.tile([P, TOK], dt.bfloat16, tag="sg")
                    nc.scalar.activation(
                        sg[:, :tok], pgs[:, :tok],
                        mybir.ActivationFunctionType.Silu,
                    )
                    nc.vector.tensor_mul(
                        h2_sb[:, m, :tok], sg[:, :tok], pus[:, :tok]
                    )

                for m2 in range(KD):
                    py = psA.tile([P, TOK], dt.float32, tag="ps", name=f"py{off}_{m2}")
                    for k2 in range(KH):
                        nc.tensor.matmul(
                            py[:, :tok], wd_sb[m2][:, k2], h2_sb[:, k2, :tok],
                            start=(k2 == 0), stop=(k2 == KH - 1),
                        )
                    ot = opool.tile([P, TOK], dt.float32, tag="ot")
                    nc.vector.tensor_copy(ot[:, :tok], py[:, :tok])
                    nc.sync.dma_start(yt[:, m2, ts], ot[:, :tok])

    return nc


def pack_lhsT(w: np.ndarray) -> np.ndarray:
    """[K, M] weight -> [p=128, m_block, k_chunk, 128] bf16, so that
    slice [:, m, k, :] is the lhsT tile for contraction chunk k, output
    block m, and each [:, m] block is one contiguous DMA."""
    K, M = w.shape
    kc, mb = K // P, M // P
    return np.ascontiguousarray(
        w.reshape(kc, P, mb, P).transpose(1, 2, 0, 3)
    ).astype(_BF16)


def pack_tokens(xe: np.ndarray, C: int) -> np.ndarray:
    """[n, D] tokens -> zero-padded [p=128, C*KD] bf16, blocked per token
    tile as [KD, tok] per partition (one contiguous DMA per tile)."""
    n = xe.shape[0]
    out = np.zeros((P, C * KD), dtype=_BF16)
    off = 0
    for tok in token_tiles(C):
        xe_t = xe[off : min(off + tok, n)]
        nt = xe_t.shape[0]
        if nt:
            blk = np.zeros((P, KD, tok), dtype=_BF16)
            # [nt, D] -> [D, nt] -> [KD, P, nt] -> [P, KD, nt]
            blk[:, :, :nt] = (
                xe_t.T.reshape(KD, P, nt).transpose(1, 0, 2).astype(_BF16)
            )
            out[:, off * KD : (off + tok) * KD] = blk.reshape(P, KD * tok)
        off += tok
    return out


def route_tokens(xf: np.ndarray, router_w: np.ndarray):
    """Top-2 routing identical to the reference (softmax over selected)."""
    logits = xf @ router_w  # [T, E]
    # top-2 per token (order irrelevant: softmax over the pair + scatter)
    top_idx = np.argpartition(-logits, TOP_K, axis=-1)[:, :TOP_K]
    tv = np.take_along_axis(logits, top_idx, axis=-1)
    tv = tv - tv.max(axis=-1, keepdims=True)
    ev = np.exp(tv)
    probs = ev / ev.sum(axis=-1, keepdims=True)

    idx, scale = [], []
    for e in range(NUM_EXPERTS):
        hit = top_idx == e  # [T, 2]
        rows = np.nonzero(hit.any(axis=-1))[0]
        w = np.where(hit[rows, 0], probs[rows, 0], probs[rows, 1])
        idx.append(rows)
        scale.append(w.astype(np.float32))
    return idx, scale


def prepare_in_maps(x, router_w, w_gate, w_up, w_down):
    x = np.asarray(x, dtype=np.float32)
    xf = x.reshape(-1, EMB)
    idx, scale = route_tokens(xf, np.asarray(router_w, dtype=np.float32))
    C = max(len(r) for r in idx)  # exact shared capacity

    in_maps = []
    for e in range(NUM_EXPERTS):
        in_maps.append(
            {
                "xt": pack_tokens(xf[idx[e]], C),
                "wg": pack_lhsT(np.asarray(w_gate[e], dtype=np.float32)),
                "wu": pack_lhsT(np.asarray(w_up[e], dtype=np.float32)),
                "wd": pack_lhsT(np.asarray(w_down[e], dtype=np.float32)),
            }
        )
    return in_maps, idx, scale, C, xf


def kernel(x, router_w, w_gate, w_up, w_down):
    from concourse.bass_utils import run_bass_kernel_spmd

    in_maps, idx, scale, C, xf = prepare_in_maps(
        x, router_w, w_gate, w_up, w_down
    )
    nc = build_moe_expert_kernel(C)
    res = None
    last_exc = None
    for _attempt in range(3):
        try:
            res = run_bass_kernel_spmd(nc, in_maps, list(range(NUM_EXPERTS)))
            break
        except Exception as exc:  # transient device wedge: retry
            last_exc = exc
    if res is None:
        raise last_exc

    out = np.zeros_like(xf)
    for e in range(NUM_EXPERTS):
        ytc = np.asarray(res.results[e]["yt"], dtype=np.float32)  # [P, KD, C]
        n = len(idx[e])
        y = ytc.transpose(1, 0, 2).reshape(EMB, C)[:, :n]  # [D, n]
        # indices within one expert are unique -> fancy += is safe
        out[idx[e]] += y.T * scale[e][:, None]
    return out.reshape(np.asarray(x).shape)


# revision 38
# speedup vs baseline: 1.0088x; 1.0000x over previous
"""MoE (top-2 of 8 experts, SwiGLU) Trainium2 kernel.

Strategy (expert parallelism, per the sharding hint):
  - Host: compute router logits/top-2/softmax (0.065% of total FLOPs),
    dispatch tokens to experts (the host-side all-to-all "dispatch").
  - Device: 8 NeuronCores, core e runs expert e's SwiGLU FFN over the
    tokens routed to it (padded to a shared capacity C). All matmuls in
    bf16 with fp32 PSUM accumulation; weights SBUF-resident.
  - Host: weighted scatter-add combine (the "combine" step).

Device compute per core (transposed so every matmul uses natural,
transpose-free operand layouts; PSUM accumulates over the contraction):
  hT[h_chunk, tok] = wg.T @ xt   (accumulate K=D over 8 chunks of 128)
  h2 = silu(hT_gate) * hT_up     (ACT silu + DVE mul, bf16 out)
  yT[d_chunk, tok] = wd.T @ h2   (accumulate K=H over 16 chunks of 128)

Weights are host-prepacked to [p=128, out_block, k_chunk, 128] so each
128-column weight block is one contiguous DMA; the per-block DMA split
lets the first matmuls start ~5us in instead of waiting ~45us for the
full 12.6MB weight load.
"""

import sys

if "/opt/trn_rl_repo" not in sys.path:
    sys.path.insert(0, "/opt/trn_rl_repo")

import ml_dtypes
import numpy as np

NUM_EXPERTS = 8
TOP_K = 2
EMB = 1024
HID = 2048
P = 128
KD = EMB // P  # 8
KH = HID // P  # 16
TOK = 512  # main token tile (one PSUM bank of f32)

_BF16 = ml_dtypes.bfloat16


def _make_tile_context(nc):
    """TileContext whose emitted instructions carry at most ONE sem wait.

    The walrus codegen bundled in this environment rejects any instruction
    with more than one sync-wait command ("Too many sync wait commands").
    Tile's scheduler freely attaches several waits to one instruction (and
    its exit drain waits on every frontier semaphore), so hoist all but the
    last wait onto dedicated same-engine NoOps immediately preceding the
    instruction.
    """
    import concourse.mybir as mybir
    import concourse.tile as tile
    from concourse.vector_clock import ScopedClock

    class OneWaitTC(tile.TileContext):
        def _split_waits(self, inst):
            si = getattr(inst, "sync_info", None)
            if si is None or not si.on_wait or len(si.on_wait) <= 1:
                return
            engine = getattr(inst, "engine", None)
            if engine is None or engine == mybir.EngineType.Unassigned:
                return
            waits = list(si.on_wait)
            for w in waits[:-1]:
                nop = mybir.InstNoOp(
                    name=self.nc.get_next_instruction_name(),
                    sync_info=mybir.SyncInfo(on_wait=[w], on_update=[]),
                    bass_nofuse=True,
                    engine=engine,
                )
                super()._commit_instruction(nop, lazy_reg_writes=False)
            inst.sync_info = mybir.SyncInfo(
                on_wait=[waits[-1]], on_update=list(si.on_update or [])
            )

        def _commit_instruction(self, inst, lazy_reg_writes: bool = True):
            if isinstance(inst, mybir.Instruction):
                self._split_waits(inst)
            super()._commit_instruction(inst, lazy_reg_writes)

        def _drain_and_barrier(self, tick_clock, wait_clock):
            nc = self.nc
            drain_inst = nc.sync.drain()
            wait_clock.add_sem_waits(
                drain_inst.ins, ScopedClock({None: tick_clock.global_clock})
            )
            si = drain_inst.ins.sync_info
            if si is not None and si.on_wait and len(si.on_wait) > 1:
                waits = list(si.on_wait)
                drain_inst.ins.sync_info = mybir.SyncInfo(
                    on_wait=waits[:1], on_update=list(si.on_update or [])
                )
                # spread the remaining frontier waits across engines so they
                # retire in parallel instead of serializing on SP
                engines = [nc.sync, nc.tensor, nc.vector, nc.scalar, nc.gpsimd]
                for i, w in enumerate(waits[1:]):
                    d2 = engines[i % len(engines)].drain()
                    d2.ins.sync_info = mybir.SyncInfo(on_wait=[w], on_update=[])
            nc.all_engine_barrier()
            assert self.sems is not None
            popped = nc._tile_sem_poison_stack.pop()
            assert popped is self._sem_poison
            nc.clear_and_free_semaphores(list(self.sems.allocated().values()))
            nc.all_engine_barrier()

    return OneWaitTC(nc)


def token_tiles(C: int):
    tiles = [TOK] * (C // TOK)
    if C % TOK:
        tiles.append(C % TOK)
    return tiles


def build_moe_expert_kernel(C: int):
    """One SPMD program: SwiGLU FFN of a single expert over C tokens."""
    import concourse.bass as bass
    import concourse.mybir as mybir

    dt = mybir.dt
    nc = bass.Bass()

    # prepacked layouts (see pack_* helpers below); xt is packed per token
    # tile ([P, KD*tok] blocks) so each tile's DMA is one contiguous
    # 8KB-per-partition read instead of 8 strided 1KB lines
    xt = nc.dram_tensor("xt", [P, C * KD], dt.bfloat16, kind="ExternalInput")
    wg = nc.dram_tensor("wg", [P, KH, KD, P], dt.bfloat16, kind="ExternalInput")
    wu = nc.dram_tensor("wu", [P, KH, KD, P], dt.bfloat16, kind="ExternalInput")
    wd = nc.dram_tensor("wd", [P, KD, KH, P], dt.bfloat16, kind="ExternalInput")
    yt = nc.dram_tensor("yt", [P, KD, C], dt.float32, kind="ExternalOutput")

    tiles = token_tiles(C)

    with _make_tile_context(nc) as tc:
        with (
            tc.tile_pool(name="weights", bufs=1) as wpool,
            tc.tile_pool(name="xin", bufs=3) as xpool,
            tc.tile_pool(name="h2", bufs=2) as hpool,
            tc.tile_pool(name="sg", bufs=4) as spool,
            tc.tile_pool(name="out", bufs=4) as opool,
            tc.tile_pool(name="psA", bufs=3, space="PSUM") as psA,
            tc.tile_pool(name="psB", bufs=2, space="PSUM") as psB,
        ):
            # one tile per 128-col weight block: tiles are Tile's dependency
            # unit, so the m=0 matmuls only wait for their own 512KB block
            wg_sb = [
                wpool.tile([P, KD, P], dt.bfloat16, tag=f"wg{m}", name=f"wg{m}")
                for m in range(KH)
            ]
            wu_sb = [
                wpool.tile([P, KD, P], dt.bfloat16, tag=f"wu{m}", name=f"wu{m}")
                for m in range(KH)
            ]
            wd_sb = [
                wpool.tile([P, KH, P], dt.bfloat16, tag=f"wd{m2}", name=f"wd{m2}")
                for m2 in range(KD)
            ]
            # token loads ride the (otherwise idle) GpSimd SWDGE path so they
            # never queue behind the 12.6MB of weights on the SP HWDGE ring.
            # The first tile is the kernel's critical path: it goes on the
            # empty ACT HWDGE ring (~4us for 1MB vs ~10us on SWDGE); only the
            # first — later ACT DMAs would stall silu work on pool-slot waits.
            xt_tiles = []
            off = 0
            for t_i, tok in enumerate(tiles):
                src = xt[:, off * KD : (off + tok) * KD].rearrange(
                    "p (k t) -> p k t", k=KD
                )
                if t_i == 0:
                    # separate half-tiles on the ACT ring: tiles are the
                    # dependency unit, so the k<4 matmuls of the first group
                    # only wait for the first 512KB half
                    half = KD // 2
                    xt0a = xpool.tile(
                        [P, half, TOK], dt.bfloat16, tag="xt0a", name="xt0a"
                    )
                    xt0b = xpool.tile(
                        [P, KD - half, TOK], dt.bfloat16, tag="xt0b", name="xt0b"
                    )
                    nc.scalar.dma_start(xt0a[:, :, :tok], src[:, :half])
                    nc.scalar.dma_start(xt0b[:, :, :tok], src[:, half:])
                    xt_tiles.append((xt0a, xt0b))
                else:
                    xt_sb = xpool.tile(
                        [P, KD, TOK], dt.bfloat16, tag="xt", name=f"xt{off}"
                    )
                    nc.gpsimd.dma_start(xt_sb[:, :, :tok], src)
                    xt_tiles.append(xt_sb)
                off += tok

            for m in range(KH):
                nc.sync.dma_start(wg_sb[m][:], wg[:, m])
                nc.sync.dma_start(wu_sb[m][:], wu[:, m])
            for m2 in range(KD):
                nc.sync.dma_start(wd_sb[m2][:], wd[:, m2])

            off = 0
            for t_i, tok in enumerate(tiles):
                ts = slice(off, off + tok)
                off += tok
                xt_sb = xt_tiles[t_i]
                if t_i == 0:
                    a, b = xt_sb
                    half = a.shape[1]
                    rhs = lambda k: (a[:, k] if k < half else b[:, k - half])
                else:
                    rhs = lambda k, _x=xt_sb: _x[:, k]

                h2_sb = hpool.tile([P, KH, TOK], dt.bfloat16, tag="h2")
                for m in range(KH):
                    pg = psA.tile([P, TOK], dt.float32, tag="pg", name=f"pg{off}_{m}")
                    for k in range(KD):
                        nc.tensor.matmul(
                            pg[:, :tok], wg_sb[m][:, k], rhs(k)[:, :tok],
                            start=(k == 0), stop=(k == KD - 1),
                        )
                    pu = psA.tile([P, TOK], dt.float32, tag="pu", name=f"pu{off}_{m}")
                    for k in range(KD):
                        nc.tensor.matmul(
                            pu[:, :tok], wu_sb[m][:, k], rhs(k)[:, :tok],
                            start=(k == 0), stop=(k == KD - 1),
                        )
                    # fast DVE copies release the PSUM banks immediately;
                    # silu+mul then run off SBUF, off the bank-recycle path
                    pgs = spool.tile([P, TOK], dt.float32, tag="pgs")
                    nc.vector.tensor_copy(pgs[:, :tok], pg[:, :tok])
                    pus = spool.tile([P, TOK], dt.float32, tag="pus")
                    nc.vector.tensor_copy(pus[:, :tok], pu[:, :tok])
                    sg = spool.tile([P, TOK], dt.bfloat16, tag="sg")
                    nc.scalar.activation(
                        sg[:, :tok], pgs[:, :tok],
                        mybir.ActivationFunctionType.Silu,
                    )
                    nc.vector.tensor_mul(
                        h2_sb[:, m, :tok], sg[:, :tok], pus[:, :tok]
                    )

                for m2 in range(KD):
                    py = psB.tile([P, TOK], dt.float32, tag="py", name=f"py{off}_{m2}")
                    for k2 in range(KH):
                        nc.tensor.matmul(
                            py[:, :tok], wd_sb[m2][:, k2], h2_sb[:, k2, :tok],
                            start=(k2 == 0), stop=(k2 == KH - 1),
                        )
                    ot = opool.tile([P, TOK], dt.float32, tag="ot")
                    nc.vector.tensor_copy(ot[:, :tok], py[:, :tok])
                    nc.sync.dma_start(yt[:, m2, ts], ot[:, :tok])

    return nc


def pack_lhsT(w: np.ndarray) -> np.ndarray:
    """[K, M] weight -> [p=128, m_block, k_chunk, 128] bf16, so that
    slice [:, m, k, :] is the lhsT tile for contraction chunk k, output
    block m, and each [:, m] block is one contiguous DMA."""
    K, M = w.shape
    kc, mb = K // P, M // P
    return np.ascontiguousarray(
        w.reshape(kc, P, mb, P).transpose(1, 2, 0, 3)
    ).astype(_BF16)


def pack_tokens(xe: np.ndarray, C: int) -> np.ndarray:
    """[n, D] tokens -> zero-padded [p=128, C*KD] bf16, blocked per token
    tile as [KD, tok] per partition (one contiguous DMA per tile)."""
    n = xe.shape[0]
    out = np.zeros((P, C * KD), dtype=_BF16)
    off = 0
    for tok in token_tiles(C):
        xe_t = xe[off : min(off + tok, n)]
        nt = xe_t.shape[0]
        if nt:
            blk = np.zeros((P, KD, tok), dtype=_BF16)
            # [nt, D] -> [D, nt] -> [KD, P, nt] -> [P, KD, nt]
            blk[:, :, :nt] = (
                xe_t.T.reshape(KD, P, nt).transpose(1, 0, 2).astype(_BF16)
            )
            out[:, off * KD : (off + tok) * KD] = blk.reshape(P, KD * tok)
        off += tok
    return out


def route_tokens(xf: np.ndarray, router_w: np.ndarray):
    """Top-2 routing identical to the reference (softmax over selected)."""
    logits = xf @ router_w  # [T, E]
    # top-2 per token (order irrelevant: softmax over the pair + scatter)
    top_idx = np.argpartition(-logits, TOP_K, axis=-1)[:, :TOP_K]
    tv = np.take_along_axis(logits, top_idx, axis=-1)
    tv = tv - tv.max(axis=-1, keepdims=True)
    ev = np.exp(tv)
    probs = ev / ev.sum(axis=-1, keepdims=True)

    idx, scale = [], []
    for e in range(NUM_EXPERTS):
        hit = top_idx == e  # [T, 2]
        rows = np.nonzero(hit.any(axis=-1))[0]
        w = np.where(hit[rows, 0], probs[rows, 0], probs[rows, 1])
        idx.append(rows)
        scale.append(w.astype(np.float32))
    return idx, scale


def prepare_in_maps(x, router_w, w_gate, w_up, w_down):
    x = np.asarray(x, dtype=np.float32)
    xf = x.reshape(-1, EMB)
    idx, scale = route_tokens(xf, np.asarray(router_w, dtype=np.float32))
    C = max(len(r) for r in idx)  # exact shared capacity

    in_maps = []
    for e in range(NUM_EXPERTS):
        in_maps.append(
            {
                "xt": pack_tokens(xf[idx[e]], C),
                "wg": pack_lhsT(np.asarray(w_gate[e], dtype=np.float32)),
                "wu": pack_lhsT(np.asarray(w_up[e], dtype=np.float32)),
                "wd": pack_lhsT(np.asarray(w_down[e], dtype=np.float32)),
            }
        )
    return in_maps, idx, scale, C, xf


def kernel(x, router_w, w_gate, w_up, w_down):
    from concourse.bass_utils import run_bass_kernel_spmd

    in_maps, idx, scale, C, xf = prepare_in_maps(
        x, router_w, w_gate, w_up, w_down
    )
    nc = build_moe_expert_kernel(C)
    res = None
    last_exc = None
    for _attempt in range(3):
        try:
            res = run_bass_kernel_spmd(nc, in_maps, list(range(NUM_EXPERTS)))
            break
        except Exception as exc:  # transient device wedge: retry
            last_exc = exc
    if res is None:
        raise last_exc

    out = np.zeros_like(xf)
    for e in range(NUM_EXPERTS):
        ytc = np.asarray(res.results[e]["yt"], dtype=np.float32)  # [P, KD, C]
        n = len(idx[e])
        y = ytc.transpose(1, 0, 2).reshape(EMB, C)[:, :n]  # [D, n]
        # indices within one expert are unique -> fancy += is safe
        out[idx[e]] += y.T * scale[e][:, None]
    return out.reshape(np.asarray(x).shape)
